# revision 1
# baseline (speedup 1.0000x reference)
"""DeepSeek-MLA attention kernel for 8 Trainium2 NeuronCores.

Sharding: tensor-parallel over heads (4 of 16 per core) x data-parallel over
batch (1 of 2 per core-group).  Core c handles batch c//4, heads
[4*(c%4), 4*(c%4)+4).  Each core computes a partial [HID, TOK] output (its
heads' contribution through Wo); the host sums the 4 partials per batch and
transposes back.

All matmuls run as float32r (TF32-like, 1 cycle/row at N>=512).  Attention uses
transposed scores sT[k, q] so softmax sums ride on the PE (ones-row matmul) and
no on-chip transposes are needed anywhere.  RoPE is applied with a
pair-interleaved row permutation so rotate_half becomes a 32-lane
stream_shuffle on the vector engine.
"""

import math
import sys

import numpy as np

for _p in ("/opt/trn_rl_repo", "/root/.axon_site/_ro/trn_rl_repo"):
    if _p not in sys.path:
        sys.path.append(_p)

# Problem dims (hardcoded per contract)
B, S, HID = 2, 2048, 2048
H, DN, DR, DV, R = 16, 128, 64, 128, 512
QHD = DN + DR  # 192
EPS = 1e-5
N_CORES = 8
NH = 4                 # heads per core
TOK = S                # tokens per core (one batch)
QROWS = NH * QHD       # 768 = 4*128 nope + 2*128 packed rope
NEG = -30000.0

_CACHE = {}

_SHUF_MASK = []
for _i in range(16):
    _SHUF_MASK += [2 * _i + 1, 2 * _i]


# ----------------------------------------------------------------------------
# Device program
# ----------------------------------------------------------------------------

def _build(causal: bool):
    import concourse.mybir as mybir
    import concourse.tile as tile
    from concourse import bacc

    F32 = mybir.dt.float32
    F32R = mybir.dt.float32r
    Exp = mybir.ActivationFunctionType.Exp
    Sqrt = mybir.ActivationFunctionType.Sqrt

    nc = bacc.Bacc("TRN2", target_bir_lowering=False, debug=False,
                   enable_asserts=False, num_devices=N_CORES)

    hT = nc.dram_tensor("hT", [HID, TOK], F32R, kind="ExternalInput").ap()
    wqT = nc.dram_tensor("wqT", [HID, QROWS], F32R, kind="ExternalInput").ap()
    wdT = nc.dram_tensor("wdT", [HID, R], F32R, kind="ExternalInput").ap()
    wuT = nc.dram_tensor("wuT", [R, QROWS], F32R, kind="ExternalInput").ap()
    wvT = nc.dram_tensor("wvT", [R, NH * DV], F32R, kind="ExternalInput").ap()
    woT = nc.dram_tensor("woT", [NH * DV, HID], F32R, kind="ExternalInput").ap()
    cosd = nc.dram_tensor("cosd", [128, TOK], F32, kind="ExternalInput").ap()
    sind = nc.dram_tensor("sind", [128, TOK], F32, kind="ExternalInput").ap()
    if causal:
        dmaskd = nc.dram_tensor("dmaskd", [128, 4 * 512], F32, kind="ExternalInput").ap()
    else:
        maskTd = nc.dram_tensor("maskTd", [S, S], F32, kind="ExternalInput").ap()
    outd = nc.dram_tensor("out", [HID, TOK], F32, kind="ExternalOutput").ap()

    with tile.TileContext(nc) as tc:
        with nc.allow_low_precision(reason="float32r is bitwise float32"):
            _emit(tc, nc, F32, F32R, Exp, Sqrt, causal,
                  hT, wqT, wdT, wuT, wvT, woT, cosd, sind,
                  dmaskd if causal else maskTd, outd)
    nc.compile()
    return nc


def _emit(tc, nc, F32, F32R, Exp, Sqrt, causal,
          hT, wqT, wdT, wuT, wvT, woT, cosd, sind, maskd, outd):
    # --- long-lived pools; LIFO release order per side is enforced ---
    pp = tc.alloc_tile_pool(name="small", bufs=1)
    ones_col = pp.tile([128, 1], F32R, name="ones_col", tag="ones_col")
    nc.vector.memset(ones_col[:].bitcast(F32), 1.0)
    ones_row = pp.tile([1, 128], F32R, name="ones_row", tag="ones_row")
    nc.vector.memset(ones_row[:].bitcast(F32), 1.0)

    pq = tc.alloc_tile_pool(name="qpool", bufs=1)
    qn = [pq.tile([128, TOK], F32R, name=f"qn{h}", tag=f"qn{h}") for h in range(4)]
    qr = [pq.tile([128, TOK], F32R, name=f"qr{p}", tag=f"qr{p}") for p in range(2)]

    pckv = tc.alloc_tile_pool(name="ckvpool", bufs=1)
    cv = [pckv.tile([128, TOK], F32R, name=f"cv{i}", tag=f"cv{i}") for i in range(4)]
    cvf = [t[:].bitcast(F32) for t in cv]   # f32 views for vector-engine reads
    # after P2 the cv tiles hold the RMS-normalized latent (in-place)
    c_nrm = [t[:] for t in cv]

    pcs = tc.alloc_tile_pool(name="cspool", bufs=1)
    cos_t = pcs.tile([128, TOK], F32, name="cos_t", tag="cos_t")
    sin_t = pcs.tile([128, TOK], F32, name="sin_t", tag="sin_t")
    nc.sync.dma_start(cos_t[:], cosd)
    nc.sync.dma_start(sin_t[:], sind)

    pqrr = tc.alloc_tile_pool(name="qrrpool", bufs=1)
    qr_raw = [pqrr.tile([128, TOK], F32, name=f"qr_raw{p}", tag=f"qr_raw{p}")
              for p in range(2)]

    # ------------------------------------------------------------------
    # P1: q / c_kv projections from hidden (streamed in token halves)
    # ------------------------------------------------------------------
    with tc.tile_pool(name="ht", bufs=16) as ph, \
         tc.tile_pool(name="wstream", bufs=4) as pw, \
         tc.tile_pool(name="p1ps", bufs=4, space="PSUM") as pps:
        groups = [
            dict(outs=[qn[0][:], qn[1][:], qn[2][:], qn[3][:]],
                 loads=[(wqT, 0, 512)], locs=[(0, 0), (0, 128), (0, 256), (0, 384)]),
            dict(outs=[qr_raw[0][:], qr_raw[1][:], cv[0][:], cv[1][:]],
                 loads=[(wqT, 512, 256), (wdT, 0, 256)],
                 locs=[(0, 0), (0, 128), (1, 0), (1, 128)]),
            dict(outs=[cv[2][:], cv[3][:]],
                 loads=[(wdT, 256, 256)], locs=[(0, 0), (0, 128)]),
        ]
        for half in range(2):
            hoff = 1024 * half
            ht = []
            for c in range(16):
                t = ph.tile([128, 1024], F32R, name=f"ht{half}_{c}", tag="ht")
                nc.sync.dma_start(t[:], hT[128 * c:128 * (c + 1), hoff:hoff + 1024])
                ht.append(t)
            for gi, g in enumerate(groups):
                pst = [pps.tile([128, 1024], F32, name=f"p1ps{half}_{gi}_{t}",
                                tag="p1ps") for t in range(len(g["outs"]))]
                for c in range(16):
                    wtl = []
                    for li, (wsrc, coff, width) in enumerate(g["loads"]):
                        wt = pw.tile([128, width], F32R,
                                     name=f"w{half}_{gi}_{c}_{li}", tag=f"wld{width}")
                        nc.sync.dma_start(
                            wt[:], wsrc[128 * c:128 * (c + 1), coff:coff + width])
                        wtl.append(wt)
                    for t in range(len(g["outs"])):
                        li, lo = g["locs"][t]
                        for mh in range(2):
                            nc.tensor.matmul(
                                pst[t][:, 512 * mh:512 * (mh + 1)],
                                wtl[li][:, lo:lo + 128],
                                ht[c][:, 512 * mh:512 * (mh + 1)],
                                start=(c == 0), stop=(c == 15))
                for t, osb in enumerate(g["outs"]):
                    nc.vector.tensor_copy(osb[:, hoff:hoff + 1024], pst[t][:])

    # RoPE on q rope tiles
    with tc.tile_pool(name="qrope", bufs=1) as pr:
        for p in range(2):
            tmp = pr.tile([128, TOK], F32, name=f"ropetmp{p}", tag="ropetmp")
            nc.vector.stream_shuffle(tmp[:], qr_raw[p][:], _SHUF_MASK)
            nc.vector.tensor_mul(tmp[:], tmp[:], sin_t[:])
            nc.vector.tensor_mul(qr_raw[p][:], qr_raw[p][:], cos_t[:])
            nc.vector.tensor_add(qr[p][:], qr_raw[p][:], tmp[:])
    pqrr.release()

    # ------------------------------------------------------------------
    # P2: RMSNorm of c_kv (norm over R on the partition axis), in place
    # ------------------------------------------------------------------
    with tc.tile_pool(name="p2tmp", bufs=2) as psq, \
         tc.tile_pool(name="p2bc", bufs=1) as pbcp, \
         tc.tile_pool(name="rows", bufs=4) as prow, \
         tc.tile_pool(name="p2ps", bufs=4, space="PSUM") as ppsr, \
         tc.tile_pool(name="p2psb", bufs=4, space="PSUM") as ppsb:
        s_bc = pbcp.tile([128, TOK], F32, name="s_bc", tag="s_bc")
        eps_t = pbcp.tile([1, 1], F32, name="eps_t", tag="eps_t")
        nc.vector.memset(eps_t[:], EPS)
        ssq_ps = [ppsr.tile([1, 512], F32, name=f"ssq{s}", tag="ssq")
                  for s in range(4)]
        for i in range(4):
            sq = psq.tile([128, TOK], F32R, name=f"sq{i}", tag="sq")
            nc.vector.tensor_mul(sq[:], cvf[i], cvf[i])
            for s in range(4):
                nc.tensor.matmul(ssq_ps[s][:], ones_col[:],
                                 sq[:, 512 * s:512 * (s + 1)],
                                 start=(i == 0), stop=(i == 3))
        for s in range(4):
            srow = prow.tile([1, 512], F32, name=f"srow{s}", tag="srow")
            nc.scalar.activation(srow[:], ssq_ps[s][:], Sqrt,
                                 bias=eps_t[:], scale=1.0 / R)
            rrow = prow.tile([1, 512], F32R, name=f"rrow{s}", tag="rrow")
            nc.vector.reciprocal(rrow[:], srow[:])
            bc_ps = ppsb.tile([128, 512], F32, name=f"bc{s}", tag="bc")
            nc.tensor.matmul(bc_ps[:], ones_row[:], rrow[:], start=True, stop=True)
            nc.vector.tensor_copy(s_bc[:, 512 * s:512 * (s + 1)], bc_ps[:])
        for i in range(4):
            nc.vector.tensor_mul(c_nrm[i], cvf[i], s_bc[:])

    # ------------------------------------------------------------------
    # P3a: k up-projection + k RoPE
    # ------------------------------------------------------------------
    pkk = tc.alloc_tile_pool(name="kpool", bufs=1, side="right")
    kn = [pkk.tile([128, TOK], F32R, name=f"kn{h}", tag=f"kn{h}") for h in range(4)]
    kr = [pkk.tile([128, TOK], F32R, name=f"kr{p}", tag=f"kr{p}") for p in range(2)]
    pkr = tc.alloc_tile_pool(name="krrpool", bufs=1, side="right")
    kr_raw = [pkr.tile([128, TOK], F32, name=f"kr_raw{p}", tag=f"kr_raw{p}")
              for p in range(2)]

    with tc.tile_pool(name="w3a", bufs=1) as pw3, \
         tc.tile_pool(name="p3k", bufs=2, space="PSUM") as ppk:
        wu_ch = []
        for i in range(4):
            wt = pw3.tile([128, QROWS], F32R, name=f"wu{i}", tag=f"wu{i}")
            nc.sync.dma_start(wt[:], wuT[128 * i:128 * (i + 1), :])
            wu_ch.append(wt)
        k_out = [(kn[0][:], 0), (kn[1][:], 128), (kn[2][:], 256), (kn[3][:], 384),
                 (kr_raw[0][:], 512), (kr_raw[1][:], 640)]
        for t, (osb, coff) in enumerate(k_out):
            ps = ppk.tile([128, TOK], F32, name=f"p3k{t}", tag="p3k")
            for i in range(4):
                for s in range(4):
                    nc.tensor.matmul(ps[:, 512 * s:512 * (s + 1)],
                                     wu_ch[i][:, coff:coff + 128],
                                     c_nrm[i][:, 512 * s:512 * (s + 1)],
                                     start=(i == 0), stop=(i == 3))
            nc.vector.tensor_copy(osb, ps[:])
    with tc.tile_pool(name="ktmp", bufs=1) as pkt:
        for p in range(2):
            tmp = pkt.tile([128, TOK], F32, name=f"kropetmp{p}", tag="kropetmp")
            nc.vector.stream_shuffle(tmp[:], kr_raw[p][:], _SHUF_MASK)
            nc.vector.tensor_mul(tmp[:], tmp[:], sin_t[:])
            nc.vector.tensor_mul(kr_raw[p][:], kr_raw[p][:], cos_t[:])
            nc.vector.tensor_add(kr[p][:], kr_raw[p][:], tmp[:])
    pkr.release()
    pcs.release()

    # ------------------------------------------------------------------
    # P3b: v up-projection
    # ------------------------------------------------------------------
    pkv = tc.alloc_tile_pool(name="vpool", bufs=1, side="right")
    v_sb = [pkv.tile([128, NH * DV], F32R, name=f"v{t}", tag=f"v{t}")
            for t in range(16)]
    with tc.tile_pool(name="w3b", bufs=1) as pwv, \
         tc.tile_pool(name="p3v", bufs=8, space="PSUM") as ppv:
        wv_ch = []
        for i in range(4):
            vt = pwv.tile([128, NH * DV], F32R, name=f"wv{i}", tag=f"wv{i}")
            nc.sync.dma_start(vt[:], wvT[128 * i:128 * (i + 1), :])
            wv_ch.append(vt)
        for tt in range(16):
            vps = ppv.tile([128, NH * DV], F32, name=f"p3v{tt}", tag="p3v")
            for i in range(4):
                nc.tensor.matmul(vps[:],
                                 c_nrm[i][:, 128 * tt:128 * (tt + 1)],
                                 wv_ch[i][:],
                                 start=(i == 0), stop=(i == 3))
            nc.vector.tensor_copy(v_sb[tt][:], vps[:])
    pckv.release()

    # ------------------------------------------------------------------
    # P4: attention per head; transposed scores sT[k, q]
    # ------------------------------------------------------------------
    po = tc.alloc_tile_pool(name="opool", bufs=1)
    o_sb = [[po.tile([128, 512], F32R, name=f"o{h}_{j}", tag=f"o{h}_{j}")
             for j in range(4)] for h in range(4)]

    pm = None if causal else tc.alloc_tile_pool(name="mload", bufs=4)
    with tc.tile_pool(name="dmask", bufs=1) as pdm, \
         tc.tile_pool(name="exp", bufs=4) as pe_, \
         tc.tile_pool(name="norm", bufs=4) as pn, \
         tc.tile_pool(name="qkps", bufs=2, space="PSUM") as pqk, \
         tc.tile_pool(name="pvps", bufs=2, space="PSUM") as ppv4, \
         tc.tile_pool(name="sumps", bufs=2, space="PSUM") as psum4, \
         tc.tile_pool(name="bcps", bufs=2, space="PSUM") as pbc4:
        if causal:
            dmask_t = pdm.tile([128, 4 * 512], F32, name="dmask_t", tag="dmask_t")
            nc.sync.dma_start(dmask_t[:], maskd)
        for h in range(4):
            p = h // 2
            rs0 = 64 * (h % 2)
            for j in range(4):
                nch = 4 * (j + 1) if causal else 16
                pv_ps = ppv4.tile([128, 512], F32, name=f"pv{h}_{j}", tag="pv")
                sm_ps = psum4.tile([1, 512], F32, name=f"sm{h}_{j}", tag="sm")
                for ci in range(nch):
                    c = ci
                    qk_ps = pqk.tile([128, 512], F32, name=f"qk{h}_{j}_{c}", tag="qk")
                    nc.tensor.matmul(qk_ps[:],
                                     kn[h][:, 128 * c:128 * (c + 1)],
                                     qn[h][:, 512 * j:512 * (j + 1)],
                                     start=True, stop=False)
                    nc.tensor.matmul(qk_ps[:],
                                     kr[p][rs0:rs0 + 64, 128 * c:128 * (c + 1)],
                                     qr[p][rs0:rs0 + 64, 512 * j:512 * (j + 1)],
                                     start=False, stop=True)
                    if causal:
                        d = c - 4 * j
                        if d >= 0:
                            nc.vector.tensor_add(
                                qk_ps[:], qk_ps[:],
                                dmask_t[:, 512 * d:512 * (d + 1)])
                    else:
                        mt = pm.tile([128, 512], F32, name=f"mt{h}{j}{c}", tag="mt")
                        nc.sync.dma_start(
                            mt[:], maskd[128 * c:128 * (c + 1),
                                         512 * j:512 * (j + 1)])
                        nc.vector.tensor_add(qk_ps[:], qk_ps[:], mt[:])
                    e = pe_.tile([128, 512], F32R, name=f"e{h}{j}{c}", tag="e")
                    nc.scalar.activation(e[:], qk_ps[:], Exp)
                    nc.tensor.matmul(pv_ps[:],
                                     v_sb[c][:, 128 * h:128 * (h + 1)],
                                     e[:],
                                     start=(ci == 0), stop=(ci == nch - 1))
                    nc.tensor.matmul(sm_ps[:], ones_col[:], e[:],
                                     start=(ci == 0), stop=(ci == nch - 1))
                rr = pn.tile([1, 512], F32R, name=f"rr{h}{j}", tag="rr")
                nc.vector.reciprocal(rr[:], sm_ps[:])
                bc_ps = pbc4.tile([128, 512], F32, name=f"abc{h}{j}", tag="abc")
                nc.tensor.matmul(bc_ps[:], ones_row[:], rr[:], start=True, stop=True)
                rbc = pn.tile([128, 512], F32, name=f"rbc{h}{j}", tag="rbc")
                nc.vector.tensor_copy(rbc[:], bc_ps[:])
                nc.vector.tensor_mul(o_sb[h][j][:], pv_ps[:], rbc[:])
    if pm is not None:
        pm.release()
    pkv.release()
    pkk.release()

    # ------------------------------------------------------------------
    # P5: output projection (partial over this core's heads)
    # ------------------------------------------------------------------
    with tc.tile_pool(name="w5", bufs=1) as pw5, \
         tc.tile_pool(name="fout", bufs=3) as pf, \
         tc.tile_pool(name="p5ps", bufs=2, space="PSUM") as pps5:
        wo_ch = []
        for i in range(4):
            wt = pw5.tile([128, HID], F32R, name=f"wo{i}", tag=f"wo{i}")
            nc.sync.dma_start(wt[:], woT[128 * i:128 * (i + 1), :])
            wo_ch.append(wt)
        for dt in range(16):
            ps = pps5.tile([128, TOK], F32, name=f"p5_{dt}", tag="p5")
            for i in range(4):
                for j in range(4):
                    nc.tensor.matmul(ps[:, 512 * j:512 * (j + 1)],
                                     wo_ch[i][:, 128 * dt:128 * (dt + 1)],
                                     o_sb[i][j][:],
                                     start=(i == 0), stop=(i == 3))
            fo = pf.tile([128, TOK], F32, name=f"fo{dt}", tag="fo")
            nc.vector.tensor_copy(fo[:], ps[:])
            nc.sync.dma_start(outd[128 * dt:128 * (dt + 1), :], fo[:])
    po.release()
    pq.release()
    pp.release()


# ----------------------------------------------------------------------------
# Host-side input preparation
# ----------------------------------------------------------------------------

_ROPE_PERM = np.empty(DR, dtype=np.int64)
_ROPE_PERM[0::2] = np.arange(32)
_ROPE_PERM[1::2] = np.arange(32, 64)


def _reorder_headsT(w_shard):
    """[NH*QHD, X] head-major rows -> [X, QROWS] transposed, nope/rope-packed."""
    blocks = []
    for h in range(NH):
        rows = w_shard[h * QHD:(h + 1) * QHD]
        blocks.append(rows[:DN])
    for pair in range(2):
        for h in (2 * pair, 2 * pair + 1):
            rows = w_shard[h * QHD:(h + 1) * QHD]
            blocks.append(rows[DN:][_ROPE_PERM])
    w_re = np.concatenate(blocks, axis=0)  # [768, X]
    return np.ascontiguousarray(w_re.T).astype(np.float32)


def _build_dmask():
    dm = np.zeros((128, 4 * 512), dtype=np.float32)
    for d in range(4):
        for m in range(4):
            blk = dm[:, 512 * d + 128 * m: 512 * d + 128 * (m + 1)]
            if m < d:
                blk[:] = NEG
            elif m == d:
                kk = np.arange(128)[:, None]
                qq = np.arange(128)[None, :]
                blk[:] = np.where(kk > qq, NEG, 0.0)
    return dm


def _is_causal(mask):
    m = np.asarray(mask).reshape(S, S)
    iu = np.triu_indices(S, 1)
    if not np.all(m[iu] <= -1e8):
        return False
    il = np.tril_indices(S)
    return bool(np.all(m[il] == 0.0))


def _prep_in_maps(inputs):
    hidden = np.ascontiguousarray(np.asarray(inputs["hidden_states"], dtype=np.float32))
    mask = np.asarray(inputs["attention_mask"], dtype=np.float32)
    position_ids = np.asarray(inputs["position_ids"]).astype(np.int64)
    Wq = np.asarray(inputs["Wq"], dtype=np.float32)
    Wkv_down = np.asarray(inputs["Wkv_down"], dtype=np.float32)
    kv_norm_w = np.asarray(inputs["kv_norm_w"], dtype=np.float32)
    Wkv_up = np.asarray(inputs["Wkv_up"], dtype=np.float32)
    Wkv_v = np.asarray(inputs["Wkv_v"], dtype=np.float32)
    Wo = np.asarray(inputs["Wo"], dtype=np.float32)
    cos = np.asarray(inputs["cos"], dtype=np.float32)
    sin = np.asarray(inputs["sin"], dtype=np.float32)

    causal = _is_causal(mask)

    pos = position_ids.reshape(-1)[:S]
    cos_g = cos[pos]                      # [S, 64]
    sin_g = sin[pos]
    cosT = np.tile(np.ascontiguousarray(cos_g.T)[_ROPE_PERM], (2, 1)).astype(np.float32)
    sinP = np.ascontiguousarray(sin_g.T)[_ROPE_PERM].copy()
    sinP[0::2] = -sinP[0::2]              # row 2i (pairs with d+32): -sin
    sinT = np.tile(sinP, (2, 1)).astype(np.float32)

    wdT = np.ascontiguousarray(Wkv_down.T).astype(np.float32)
    Wkv_up_w = Wkv_up * kv_norm_w[None, :]
    Wkv_v_w = Wkv_v * kv_norm_w[None, :]

    dmask = _build_dmask() if causal else None
    maskT = None if causal else np.ascontiguousarray(mask.reshape(S, S).T)

    in_maps = []
    for c in range(N_CORES):
        b, g = divmod(c, 4)
        heads = slice(g * NH * QHD, (g + 1) * NH * QHD)
        vh = slice(g * NH * DV, (g + 1) * NH * DV)
        m = {
            "hT": np.ascontiguousarray(hidden[b].T),
            "wqT": _reorder_headsT(Wq[heads] * np.float32(1.0 / math.sqrt(QHD))),
            "wdT": wdT,
            "wuT": _reorder_headsT(Wkv_up_w[heads]),
            "wvT": np.ascontiguousarray(Wkv_v_w[vh].T).astype(np.float32),
            "woT": np.ascontiguousarray(Wo[:, vh].T).astype(np.float32),
            "cosd": cosT,
            "sind": sinT,
        }
        if causal:
            m["dmaskd"] = dmask
        else:
            m["maskTd"] = maskT
        in_maps.append(m)
    return causal, in_maps


def _combine(results):
    out = np.zeros((B, S, HID), dtype=np.float32)
    for b in range(B):
        acc = results[4 * b]["out"].astype(np.float64)
        for g in range(1, 4):
            acc = acc + results[4 * b + g]["out"]
        out[b] = acc.T.astype(np.float32)
    return out


def kernel(**inputs):
    from concourse import bass_utils

    causal, in_maps = _prep_in_maps(inputs)
    if causal not in _CACHE:
        _CACHE[causal] = _build(causal)
    nc = _CACHE[causal]

    res = bass_utils.run_bass_kernel_spmd(nc, in_maps, core_ids=list(range(N_CORES)))
    return _combine(res.results)



# revision 18
# speedup vs baseline: 127.6610x; 127.6610x over previous
"""DeepSeek-MLA attention kernel for 8 Trainium2 NeuronCores.

Sharding: tensor-parallel over heads (4 of 16 per core) x data-parallel over
batch (1 of 2 per core-group).  Core c handles batch c//4, heads
[4*(c%4), 4*(c%4)+4).  Each core computes a partial [HID, TOK] output (its
heads' contribution through Wo); the host sums the 4 partials per batch and
transposes back.

All matmuls run as float32r.  Attention uses transposed scores sT[k, q] so
softmax sums ride on the PE (ones-row matmul).  RoPE is applied with a
pair-interleaved row permutation so rotate_half becomes a 32-lane
stream_shuffle on the vector engine, reading straight out of PSUM.

Scheduling notes (causal fast path):
- weight/const DMAs ride the scalar-engine HWDGE queue; hT and output DMAs
  ride sync, so neither stream queues behind the other.
- P1 runs groups g1(cv01+qrope), g2(cv23), g0(qn) per token-half; RoPE and
  the RMS-norm squares overlap the remaining P1 matmuls.
- P4 interleaves head pairs (2 chains) so the in-order PE queue never waits
  on the scalar-engine exp; the causal mask is added on the PE via an
  identity matmul; softmax normalization uses reciprocal_approx_fast.
"""

import math
import sys

import numpy as np

for _p in ("/opt/trn_rl_repo", "/root/.axon_site/_ro/trn_rl_repo"):
    if _p not in sys.path:
        sys.path.append(_p)

# Problem dims (hardcoded per contract)
B, S, HID = 2, 2048, 2048
H, DN, DR, DV, R = 16, 128, 64, 128, 512
QHD = DN + DR  # 192
EPS = 1e-5
N_CORES = 8
NH = 4                 # heads per core
TOK = S                # tokens per core (one batch)
QROWS = NH * QHD       # 768 = 4*128 nope + 2*128 packed rope
NEG = -30000.0

_CACHE = {}

_SHUF_MASK = []
for _i in range(16):
    _SHUF_MASK += [2 * _i + 1, 2 * _i]


# ----------------------------------------------------------------------------
# Device program (causal fast path)
# ----------------------------------------------------------------------------

def _build(causal: bool):
    import concourse.mybir as mybir
    import concourse.tile as tile
    from concourse import bacc

    F32 = mybir.dt.float32
    F32R = mybir.dt.float32r

    nc = bacc.Bacc("TRN2", target_bir_lowering=False, debug=False,
                   enable_asserts=False, num_devices=N_CORES)

    hT = nc.dram_tensor("hT", [HID, TOK], F32R, kind="ExternalInput").ap()
    cosd = nc.dram_tensor("cosd", [128, TOK], F32, kind="ExternalInput").ap()
    sind = nc.dram_tensor("sind", [128, TOK], F32, kind="ExternalInput").ap()
    wuT = nc.dram_tensor("wuT", [R, QROWS], F32R, kind="ExternalInput").ap()
    wvT = nc.dram_tensor("wvT", [R, NH * DV], F32R, kind="ExternalInput").ap()
    woT = nc.dram_tensor("woT", [NH * DV, HID], F32R, kind="ExternalInput").ap()
    outd = nc.dram_tensor("out", [HID, TOK], F32, kind="ExternalOutput").ap()

    if causal:
        w0T = nc.dram_tensor("w0T", [HID, 512], F32R, kind="ExternalInput").ap()
        w1T = nc.dram_tensor("w1T", [HID, 512], F32R, kind="ExternalInput").ap()
        w2T = nc.dram_tensor("w2T", [HID, 256], F32R, kind="ExternalInput").ap()
        identd = nc.dram_tensor("identd", [128, 128], F32R, kind="ExternalInput").ap()
        dmaskd = nc.dram_tensor("dmaskd", [128, 4 * 512], F32R, kind="ExternalInput").ap()
        with tile.TileContext(nc) as tc:
            with nc.allow_low_precision(reason="float32r is bitwise float32"):
                _emit_fast(tc, nc, mybir, hT, w0T, w1T, w2T, wuT, wvT, woT,
                           cosd, sind, identd, dmaskd, outd)
    else:
        wqT = nc.dram_tensor("wqT", [HID, QROWS], F32R, kind="ExternalInput").ap()
        wdT = nc.dram_tensor("wdT", [HID, R], F32R, kind="ExternalInput").ap()
        maskTd = nc.dram_tensor("maskTd", [S, S], F32, kind="ExternalInput").ap()
        with tile.TileContext(nc) as tc:
            with nc.allow_low_precision(reason="float32r is bitwise float32"):
                _emit_old(tc, nc, mybir, hT, wqT, wdT, wuT, wvT, woT,
                          cosd, sind, maskTd, outd)
    nc.compile()
    return nc


def _emit_fast(tc, nc, mybir, hT, w0T, w1T, w2T, wuT, wvT, woT,
               cosd, sind, identd, dmaskd, outd):
    F32 = mybir.dt.float32
    F32R = mybir.dt.float32r
    BF16 = mybir.dt.bfloat16
    Exp = mybir.ActivationFunctionType.Exp
    Sqrt = mybir.ActivationFunctionType.Sqrt
    Copy = mybir.ActivationFunctionType.Copy

    # --- long-lived pools ---
    pp = tc.alloc_tile_pool(name="small", bufs=1)
    ones_col = pp.tile([128, 1], F32R, name="ones_col", tag="ones_col")
    nc.vector.memset(ones_col[:].bitcast(F32), 1.0)
    ones_row = pp.tile([1, 128], F32R, name="ones_row", tag="ones_row")
    nc.vector.memset(ones_row[:].bitcast(F32), 1.0)
    ones_col_bf = pp.tile([128, 1], BF16, name="ones_col_bf", tag="ones_col_bf")
    nc.vector.memset(ones_col_bf[:], 1.0)
    eps_t = pp.tile([1, 1], F32, name="eps_t", tag="eps_t")
    nc.vector.memset(eps_t[:], EPS)
    ident = pp.tile([128, 128], F32R, name="ident", tag="ident")
    nc.scalar.dma_start(ident[:], identd)

    pq = tc.alloc_tile_pool(name="qpool", bufs=1)
    qn = [pq.tile([128, TOK], F32R, name=f"qn{h}", tag=f"qn{h}") for h in range(4)]
    qr = [pq.tile([128, TOK], F32R, name=f"qr{p}", tag=f"qr{p}") for p in range(2)]

    pckv = tc.alloc_tile_pool(name="ckvpool", bufs=1)
    cv = [pckv.tile([128, TOK], F32R, name=f"cv{i}", tag=f"cv{i}") for i in range(4)]
    cvf = [t[:].bitcast(F32) for t in cv]
    c_nrm = [t[:] for t in cv]   # after P2 the cv tiles hold normalized latent

    pcs = tc.alloc_tile_pool(name="cspool", bufs=1)
    cos_t = pcs.tile([128, TOK], F32, name="cos_t", tag="cos_t")
    sin_t = pcs.tile([128, TOK], F32, name="sin_t", tag="sin_t")
    nc.scalar.dma_start(cos_t[:], cosd)
    nc.scalar.dma_start(sin_t[:], sind)

    # P3 weight pool: allocated early (LIFO stacking), DMA-triggered after P1
    pw3 = tc.alloc_tile_pool(name="w3a", bufs=1)
    wu_ch = [pw3.tile([128, QROWS], F32R, name=f"wu{i}", tag=f"wu{i}")
             for i in range(4)]
    wv_ch = [pw3.tile([128, NH * DV], F32R, name=f"wv{i}", tag=f"wv{i}")
             for i in range(4)]

    # ------------------------------------------------------------------
    # P1: q / c_kv projections from hidden (streamed in token halves).
    # Groups per half: g1 -> [cv0, cv1, qr_ps0, qr_ps1] (w1T),
    #                  g2 -> [cv2, cv3] (w2T),  g0 -> [qn0..3] (w0T).
    # RoPE for q is applied straight out of the g1 psum tiles; the RMS-norm
    # squares (bf16) are computed per half as well.  All of that vector work
    # overlaps the g2/g0 matmuls.
    # ------------------------------------------------------------------
    psq = tc.alloc_tile_pool(name="sqpool", bufs=1)
    sq = [[psq.tile([128, 1024], BF16, name=f"sq{i}_{hf}", tag=f"sq{i}_{hf}")
           for hf in range(2)] for i in range(4)]

    with tc.tile_pool(name="ht", bufs=16) as ph, \
         tc.tile_pool(name="wstream", bufs=3) as pw, \
         tc.tile_pool(name="ropetmp", bufs=1) as prt, \
         tc.tile_pool(name="p1ps", bufs=4, space="PSUM") as pps:
        groups = [
            # (wsrc, width, outs) ; outs: list of (kind, index) kind q=qn c=cv r=qr_psum
            (w1T, 512, [("c", 0), ("c", 1), ("r", 0), ("r", 1)]),
            (w2T, 256, [("c", 2), ("c", 3)]),
            (w0T, 512, [("q", 0), ("q", 1), ("q", 2), ("q", 3)]),
        ]
        for half in range(2):
            hoff = 1024 * half
            ht = []
            for c in range(16):
                t = ph.tile([128, 1024], F32R, name=f"ht{half}_{c}", tag="ht")
                nc.sync.dma_start(t[:], hT[128 * c:128 * (c + 1), hoff:hoff + 1024])
                ht.append(t)
            for gi, (wsrc, width, outs) in enumerate(groups):
                pst = [pps.tile([128, 1024], F32, name=f"p1ps{half}_{gi}_{t}",
                                tag="p1ps") for t in range(len(outs))]
                for c in range(16):
                    wt = pw.tile([128, width], F32R,
                                 name=f"w{half}_{gi}_{c}", tag="wld")
                    nc.scalar.dma_start(
                        wt[:], wsrc[128 * c:128 * (c + 1), :])
                    for t in range(len(outs)):
                        for mh in range(2):
                            nc.tensor.matmul(
                                pst[t][:, 512 * mh:512 * (mh + 1)],
                                wt[:, 128 * t:128 * (t + 1)],
                                ht[c][:, 512 * mh:512 * (mh + 1)],
                                start=(c == 0), stop=(c == 15))
                # evacuate psum / fuse rope, and P2 squares
                for t, (kind, idx) in enumerate(outs):
                    if kind == "c":
                        nc.vector.tensor_copy(cv[idx][:][:, hoff:hoff + 1024],
                                              pst[t][:])
                        # square for RMS norm (bf16), off the sbuf copy
                        nc.vector.tensor_mul(sq[idx][half][:],
                                             cvf[idx][:, hoff:hoff + 1024],
                                             cvf[idx][:, hoff:hoff + 1024])
                    elif kind == "q":
                        nc.vector.tensor_copy(qn[idx][:][:, hoff:hoff + 1024],
                                              pst[t][:])
                    else:  # rope, fused from psum
                        p = idx
                        tmp = prt.tile([128, 1024], F32, name=f"rtmp{half}_{p}",
                                       tag="rtmp")
                        nc.vector.stream_shuffle(tmp[:], pst[t][:], _SHUF_MASK)
                        nc.vector.tensor_mul(qr[p][:][:, hoff:hoff + 1024],
                                             pst[t][:],
                                             cos_t[:, hoff:hoff + 1024])
                        nc.vector.tensor_mul(tmp[:], tmp[:],
                                             sin_t[:, hoff:hoff + 1024])
                        nc.vector.tensor_add(qr[p][:][:, hoff:hoff + 1024],
                                             qr[p][:].bitcast(F32)[:, hoff:hoff + 1024],
                                             tmp[:])

    # trigger the P3 weight prefetch now that the ht pool is gone
    for i in range(4):
        nc.scalar.dma_start(wu_ch[i][:], wuT[128 * i:128 * (i + 1), :])
    for i in range(4):
        nc.scalar.dma_start(wv_ch[i][:], wvT[128 * i:128 * (i + 1), :])

    # ------------------------------------------------------------------
    # P2: RMS-norm scale.  ssq via ones-matmul on the PE (sq tiles are
    # already in sbuf), sqrt on scalar, reciprocal_approx on vector,
    # broadcast via ones-row matmul.  c_nrm written s-block by s-block in
    # the order P3a consumes them.
    # ------------------------------------------------------------------
    with tc.tile_pool(name="p2rows", bufs=8) as prow, \
         tc.tile_pool(name="p2ps", bufs=4, space="PSUM") as ppsr, \
         tc.tile_pool(name="p2psb", bufs=2, space="PSUM") as ppsb:
        ssq_ps = [ppsr.tile([1, 512], F32, name=f"ssq{s}", tag="ssq", bufs=4)
                  for s in range(4)]
        for s in range(4):
            hf, s2 = divmod(s, 2)
            for i in range(4):
                nc.tensor.matmul(ssq_ps[s][:], ones_col_bf[:],
                                 sq[i][hf][:, 512 * s2:512 * (s2 + 1)],
                                 start=(i == 0), stop=(i == 3))
        for s in range(4):
            srow = prow.tile([1, 512], F32R, name=f"srow{s}", tag="srow")
            nc.scalar.activation(srow[:], ssq_ps[s][:], Sqrt,
                                 bias=eps_t[:], scale=1.0 / R)
            bc_ps = ppsb.tile([128, 512], F32, name=f"bc{s}", tag="bc")
            nc.tensor.matmul(bc_ps[:], ones_row[:], srow[:],
                             start=True, stop=True)
            rbc = prow.tile([128, 512], F32, name=f"sbc{s}", tag="sbc", bufs=2)
            nc.vector.reciprocal_approx_fast(rbc[:], bc_ps[:])
            for i in range(4):
                nc.vector.tensor_mul(
                    c_nrm[i][:, 512 * s:512 * (s + 1)],
                    cvf[i][:, 512 * s:512 * (s + 1)],
                    rbc[:])
    psq.release()

    # ------------------------------------------------------------------
    # P3a: k up-projection, s-block-major so it starts as soon as the first
    # c_nrm s-block lands.  kr psum tiles get RoPE applied directly; kn goes
    # through plain copies.
    # ------------------------------------------------------------------
    pkk = tc.alloc_tile_pool(name="kpool", bufs=1, side="right")
    kn = [pkk.tile([128, TOK], F32R, name=f"kn{h}", tag=f"kn{h}") for h in range(4)]
    kr = [pkk.tile([128, TOK], F32R, name=f"kr{p}", tag=f"kr{p}") for p in range(2)]

    # col offsets in wuT (reordered): kn0..3 at 0..511, kr0 at 512, kr1 at 640
    k_outs = [("r", 0, 512), ("r", 1, 640), ("n", 0, 0), ("n", 1, 128),
              ("n", 2, 256), ("n", 3, 384)]
    with tc.tile_pool(name="krt", bufs=2) as pkt, \
         tc.tile_pool(name="p3k", bufs=6, space="PSUM") as ppk:
        for s in range(4):
            soff = 512 * s
            ps = [ppk.tile([128, 512], F32, name=f"p3k{s}_{t}", tag="p3k")
                  for t in range(6)]
            for i in range(4):
                for t, (kind, idx, coff) in enumerate(k_outs):
                    nc.tensor.matmul(ps[t][:],
                                     wu_ch[i][:, coff:coff + 128],
                                     c_nrm[i][:, soff:soff + 512],
                                     start=(i == 0), stop=(i == 3))
            for t, (kind, idx, coff) in enumerate(k_outs):
                if kind == "n":
                    nc.vector.tensor_copy(
                        kn[idx][:][:, soff:soff + 512], ps[t][:])
                else:
                    p = idx
                    tmp = pkt.tile([128, 512], F32, name=f"ktmp{s}_{p}",
                                   tag="ktmp")
                    nc.vector.stream_shuffle(tmp[:], ps[t][:], _SHUF_MASK)
                    nc.vector.tensor_mul(kr[p][:][:, soff:soff + 512],
                                         ps[t][:],
                                         cos_t[:, soff:soff + 512])
                    nc.vector.tensor_mul(tmp[:], tmp[:],
                                         sin_t[:, soff:soff + 512])
                    nc.vector.tensor_add(kr[p][:][:, soff:soff + 512],
                                         kr[p][:].bitcast(F32)[:, soff:soff + 512],
                                         tmp[:])

    # ------------------------------------------------------------------
    # P3b: v up-projection (v rows = tokens, cols = head*dv)
    # ------------------------------------------------------------------
    pkv = tc.alloc_tile_pool(name="vpool", bufs=1, side="right")
    dmask_t = pkv.tile([128, 4 * 512], F32R, name="dmask_t", tag="dmask_t")
    nc.sync.dma_start(dmask_t[:], dmaskd)
    v_sb = [pkv.tile([128, NH * DV], F32R, name=f"v{t}", tag=f"v{t}")
            for t in range(16)]
    with tc.tile_pool(name="p3v", bufs=4, space="PSUM") as ppv:
        for tt in range(16):
            vps = ppv.tile([128, NH * DV], F32, name=f"p3v{tt}", tag="p3v")
            for i in range(4):
                nc.tensor.matmul(vps[:],
                                 c_nrm[i][:, 128 * tt:128 * (tt + 1)],
                                 wv_ch[i][:],
                                 start=(i == 0), stop=(i == 3))
            nc.scalar.activation(v_sb[tt][:], vps[:], Copy)
    pw3.release()
    pcs.release()
    pckv.release()

    # ------------------------------------------------------------------
    # P4: attention, head pairs interleaved.  Scores sT[k, q]; causal mask
    # added on the PE via identity matmul; exp on scalar; pv/sum accumulate
    # on the PE; normalization via reciprocal_approx_fast + ones-row
    # broadcast, deferred past the next chain's first tiles.
    # ------------------------------------------------------------------
    po = tc.alloc_tile_pool(name="opool", bufs=1)
    o_sb = [[po.tile([128, 512], F32R, name=f"o{h}_{j}", tag=f"o{h}_{j}")
             for j in range(4)] for h in range(4)]

    dmask_r = dmask_t[:]

    with tc.tile_pool(name="exp", bufs=4) as pe_, \
         tc.tile_pool(name="norm", bufs=2) as pn, \
         tc.tile_pool(name="qkps", bufs=3, space="PSUM") as pqk, \
         tc.tile_pool(name="pvps", bufs=3, space="PSUM") as ppv4, \
         tc.tile_pool(name="sumps", bufs=2, space="PSUM") as psum4:

        pending = []   # deferred end-of-chain work from the previous (hp,j)

        def flush_pe(items):
            out = []
            for (hh, jj, pv_ps, sm_ps) in items:
                smr = pn.tile([1, 512], F32R, name=f"smr{hh}{jj}", tag="smr")
                nc.vector.tensor_copy(smr[:], sm_ps[:])
                bc_ps = pqk.tile([128, 512], F32, name=f"abc{hh}{jj}", tag="qk")
                nc.tensor.matmul(bc_ps[:], ones_row[:], smr[:],
                                 start=True, stop=True)
                out.append((hh, jj, pv_ps, bc_ps))
            return out

        def flush_rest(items):
            for (hh, jj, pv_ps, bc_ps) in items:
                rbc = pn.tile([128, 512], F32, name=f"rbc{hh}{jj}", tag="rbc")
                nc.vector.reciprocal_approx_fast(rbc[:], bc_ps[:])
                nc.vector.tensor_mul(o_sb[hh][jj][:], pv_ps[:], rbc[:])

        for hp in range(2):
            hA, hB = 2 * hp, 2 * hp + 1
            p = hp
            for j in range(4):
                nch = 4 * (j + 1)
                pv_A = ppv4.tile([128, 512], F32, name=f"pv{hA}_{j}", tag="pv")
                pv_B = ppv4.tile([128, 512], F32, name=f"pv{hB}_{j}", tag="pv")
                sm_A = psum4.tile([1, 512], F32, name=f"sm{hA}_{j}", tag="sm")
                sm_B = psum4.tile([1, 512], F32, name=f"sm{hB}_{j}", tag="sm")
                eA = eB = None
                for ci in range(nch):
                    c = ci
                    d = c - 4 * j
                    # qk for chain A
                    qk_A = pqk.tile([128, 512], F32, name=f"qk{hA}_{j}_{c}",
                                    tag="qk")
                    nc.tensor.matmul(qk_A[:],
                                     kn[hA][:, 128 * c:128 * (c + 1)],
                                     qn[hA][:, 512 * j:512 * (j + 1)],
                                     start=True, stop=False)
                    nc.tensor.matmul(qk_A[:],
                                     kr[p][0:64, 128 * c:128 * (c + 1)],
                                     qr[p][0:64, 512 * j:512 * (j + 1)],
                                     start=False, stop=(d < 0))
                    if d >= 0:
                        nc.tensor.matmul(qk_A[:], ident[:],
                                         dmask_r[:, 512 * d:512 * (d + 1)],
                                         start=False, stop=True)
                    if ci > 0:
                        # chain B previous ci accumulations
                        nc.tensor.matmul(pv_B[:],
                                         v_sb[ci - 1][:, 128 * hB:128 * (hB + 1)],
                                         eB[:],
                                         start=(ci - 1 == 0), stop=(ci - 1 == nch - 1))
                        nc.tensor.matmul(sm_B[:], ones_col[:], eB[:],
                                         start=(ci - 1 == 0), stop=(ci - 1 == nch - 1))
                    eA = pe_.tile([128, 512], F32R, name=f"e{hA}{j}{c}", tag="e")
                    nc.scalar.activation(eA[:], qk_A[:], Exp)
                    # qk for chain B
                    qk_B = pqk.tile([128, 512], F32, name=f"qk{hB}_{j}_{c}",
                                    tag="qk")
                    nc.tensor.matmul(qk_B[:],
                                     kn[hB][:, 128 * c:128 * (c + 1)],
                                     qn[hB][:, 512 * j:512 * (j + 1)],
                                     start=True, stop=False)
                    nc.tensor.matmul(qk_B[:],
                                     kr[p][64:128, 128 * c:128 * (c + 1)],
                                     qr[p][64:128, 512 * j:512 * (j + 1)],
                                     start=False, stop=(d < 0))
                    if d >= 0:
                        nc.tensor.matmul(qk_B[:], ident[:],
                                         dmask_r[:, 512 * d:512 * (d + 1)],
                                         start=False, stop=True)
                    if ci == 1 and pending:
                        # deferred broadcast matmuls from the previous chain
                        pending = flush_pe(pending)
                    # chain A current ci accumulations
                    nc.tensor.matmul(pv_A[:],
                                     v_sb[ci][:, 128 * hA:128 * (hA + 1)],
                                     eA[:],
                                     start=(ci == 0), stop=(ci == nch - 1))
                    nc.tensor.matmul(sm_A[:], ones_col[:], eA[:],
                                     start=(ci == 0), stop=(ci == nch - 1))
                    if ci == 1 and pending:
                        flush_rest(pending)
                        pending = []
                    eB = pe_.tile([128, 512], F32R, name=f"e{hB}{j}{c}", tag="e")
                    nc.scalar.activation(eB[:], qk_B[:], Exp)
                # tail: chain B last ci
                nc.tensor.matmul(pv_B[:],
                                 v_sb[nch - 1][:, 128 * hB:128 * (hB + 1)],
                                 eB[:],
                                 start=(nch - 1 == 0), stop=True)
                nc.tensor.matmul(sm_B[:], ones_col[:], eB[:],
                                 start=(nch - 1 == 0), stop=True)
                pending = [(hA, j, pv_A, sm_A), (hB, j, pv_B, sm_B)]
            # flush at hp boundary so psum pools stay tidy
            pending = flush_pe(pending)
            flush_rest(pending)
            pending = []
    pkv.release()
    pkk.release()

    # ------------------------------------------------------------------
    # P5: output projection (partial over this core's heads)
    # ------------------------------------------------------------------
    with tc.tile_pool(name="w5", bufs=1) as pw5, \
         tc.tile_pool(name="fout", bufs=3) as pf, \
         tc.tile_pool(name="p5ps", bufs=2, space="PSUM") as pps5:
        wo_ch = []
        for i in range(4):
            wt = pw5.tile([128, HID], F32R, name=f"wo{i}", tag=f"wo{i}")
            nc.scalar.dma_start(wt[:], woT[128 * i:128 * (i + 1), :])
            wo_ch.append(wt)
        for dt in range(16):
            ps = pps5.tile([128, TOK], F32, name=f"p5_{dt}", tag="p5")
            for i in range(4):
                for j in range(4):
                    nc.tensor.matmul(ps[:, 512 * j:512 * (j + 1)],
                                     wo_ch[i][:, 128 * dt:128 * (dt + 1)],
                                     o_sb[i][j][:],
                                     start=(i == 0), stop=(i == 3))
            fo = pf.tile([128, TOK], F32, name=f"fo{dt}", tag="fo")
            if dt % 2 == 0:
                nc.vector.tensor_copy(fo[:], ps[:])
            else:
                nc.scalar.activation(fo[:], ps[:],
                                     mybir.ActivationFunctionType.Copy)
            nc.sync.dma_start(outd[128 * dt:128 * (dt + 1), :], fo[:])
    po.release()
    pq.release()
    pp.release()


# ----------------------------------------------------------------------------
# Old (baseline) device program, kept for the non-causal fallback
# ----------------------------------------------------------------------------

def _emit_old(tc, nc, mybir, hT, wqT, wdT, wuT, wvT, woT, cosd, sind,
              maskTd, outd):
    F32 = mybir.dt.float32
    F32R = mybir.dt.float32r
    Exp = mybir.ActivationFunctionType.Exp
    Sqrt = mybir.ActivationFunctionType.Sqrt

    pp = tc.alloc_tile_pool(name="small", bufs=1)
    ones_col = pp.tile([128, 1], F32R, name="ones_col", tag="ones_col")
    nc.vector.memset(ones_col[:].bitcast(F32), 1.0)
    ones_row = pp.tile([1, 128], F32R, name="ones_row", tag="ones_row")
    nc.vector.memset(ones_row[:].bitcast(F32), 1.0)

    pq = tc.alloc_tile_pool(name="qpool", bufs=1)
    qn = [pq.tile([128, TOK], F32R, name=f"qn{h}", tag=f"qn{h}") for h in range(4)]
    qr = [pq.tile([128, TOK], F32R, name=f"qr{p}", tag=f"qr{p}") for p in range(2)]

    pckv = tc.alloc_tile_pool(name="ckvpool", bufs=1)
    cv = [pckv.tile([128, TOK], F32R, name=f"cv{i}", tag=f"cv{i}") for i in range(4)]
    cvf = [t[:].bitcast(F32) for t in cv]
    c_nrm = [t[:] for t in cv]

    pcs = tc.alloc_tile_pool(name="cspool", bufs=1)
    cos_t = pcs.tile([128, TOK], F32, name="cos_t", tag="cos_t")
    sin_t = pcs.tile([128, TOK], F32, name="sin_t", tag="sin_t")
    nc.sync.dma_start(cos_t[:], cosd)
    nc.sync.dma_start(sin_t[:], sind)

    pqrr = tc.alloc_tile_pool(name="qrrpool", bufs=1)
    qr_raw = [pqrr.tile([128, TOK], F32, name=f"qr_raw{p}", tag=f"qr_raw{p}")
              for p in range(2)]

    with tc.tile_pool(name="ht", bufs=16) as ph, \
         tc.tile_pool(name="wstream", bufs=4) as pw, \
         tc.tile_pool(name="p1ps", bufs=4, space="PSUM") as pps:
        groups = [
            dict(outs=[qn[0][:], qn[1][:], qn[2][:], qn[3][:]],
                 loads=[(wqT, 0, 512)], locs=[(0, 0), (0, 128), (0, 256), (0, 384)]),
            dict(outs=[qr_raw[0][:], qr_raw[1][:], cv[0][:], cv[1][:]],
                 loads=[(wqT, 512, 256), (wdT, 0, 256)],
                 locs=[(0, 0), (0, 128), (1, 0), (1, 128)]),
            dict(outs=[cv[2][:], cv[3][:]],
                 loads=[(wdT, 256, 256)], locs=[(0, 0), (0, 128)]),
        ]
        for half in range(2):
            hoff = 1024 * half
            ht = []
            for c in range(16):
                t = ph.tile([128, 1024], F32R, name=f"ht{half}_{c}", tag="ht")
                nc.sync.dma_start(t[:], hT[128 * c:128 * (c + 1), hoff:hoff + 1024])
                ht.append(t)
            for gi, g in enumerate(groups):
                pst = [pps.tile([128, 1024], F32, name=f"p1ps{half}_{gi}_{t}",
                                tag="p1ps") for t in range(len(g["outs"]))]
                for c in range(16):
                    wtl = []
                    for li, (wsrc, coff, width) in enumerate(g["loads"]):
                        wt = pw.tile([128, width], F32R,
                                     name=f"w{half}_{gi}_{c}_{li}", tag=f"wld{width}")
                        nc.sync.dma_start(
                            wt[:], wsrc[128 * c:128 * (c + 1), coff:coff + width])
                        wtl.append(wt)
                    for t in range(len(g["outs"])):
                        li, lo = g["locs"][t]
                        for mh in range(2):
                            nc.tensor.matmul(
                                pst[t][:, 512 * mh:512 * (mh + 1)],
                                wtl[li][:, lo:lo + 128],
                                ht[c][:, 512 * mh:512 * (mh + 1)],
                                start=(c == 0), stop=(c == 15))
                for t, osb in enumerate(g["outs"]):
                    nc.vector.tensor_copy(osb[:, hoff:hoff + 1024], pst[t][:])

    with tc.tile_pool(name="qrope", bufs=1) as pr:
        for p in range(2):
            tmp = pr.tile([128, TOK], F32, name=f"ropetmp{p}", tag="ropetmp")
            nc.vector.stream_shuffle(tmp[:], qr_raw[p][:], _SHUF_MASK)
            nc.vector.tensor_mul(tmp[:], tmp[:], sin_t[:])
            nc.vector.tensor_mul(qr_raw[p][:], qr_raw[p][:], cos_t[:])
            nc.vector.tensor_add(qr[p][:], qr_raw[p][:], tmp[:])
    pqrr.release()

    with tc.tile_pool(name="p2tmp", bufs=2) as psq, \
         tc.tile_pool(name="p2bc", bufs=1) as pbcp, \
         tc.tile_pool(name="rows", bufs=4) as prow, \
         tc.tile_pool(name="p2ps", bufs=4, space="PSUM") as ppsr, \
         tc.tile_pool(name="p2psb", bufs=4, space="PSUM") as ppsb:
        s_bc = pbcp.tile([128, TOK], F32, name="s_bc", tag="s_bc")
        eps_t = pbcp.tile([1, 1], F32, name="eps_t", tag="eps_t")
        nc.vector.memset(eps_t[:], EPS)
        ssq_ps = [ppsr.tile([1, 512], F32, name=f"ssq{s}", tag="ssq")
                  for s in range(4)]
        for i in range(4):
            sqt = psq.tile([128, TOK], F32R, name=f"sq{i}", tag="sq")
            nc.vector.tensor_mul(sqt[:], cvf[i], cvf[i])
            for s in range(4):
                nc.tensor.matmul(ssq_ps[s][:], ones_col[:],
                                 sqt[:, 512 * s:512 * (s + 1)],
                                 start=(i == 0), stop=(i == 3))
        for s in range(4):
            srow = prow.tile([1, 512], F32, name=f"srow{s}", tag="srow")
            nc.scalar.activation(srow[:], ssq_ps[s][:], Sqrt,
                                 bias=eps_t[:], scale=1.0 / R)
            rrow = prow.tile([1, 512], F32R, name=f"rrow{s}", tag="rrow")
            nc.vector.reciprocal(rrow[:], srow[:])
            bc_ps = ppsb.tile([128, 512], F32, name=f"bc{s}", tag="bc")
            nc.tensor.matmul(bc_ps[:], ones_row[:], rrow[:], start=True, stop=True)
            nc.vector.tensor_copy(s_bc[:, 512 * s:512 * (s + 1)], bc_ps[:])
        for i in range(4):
            nc.vector.tensor_mul(c_nrm[i], cvf[i], s_bc[:])

    pkk = tc.alloc_tile_pool(name="kpool", bufs=1, side="right")
    kn = [pkk.tile([128, TOK], F32R, name=f"kn{h}", tag=f"kn{h}") for h in range(4)]
    kr = [pkk.tile([128, TOK], F32R, name=f"kr{p}", tag=f"kr{p}") for p in range(2)]
    pkr = tc.alloc_tile_pool(name="krrpool", bufs=1, side="right")
    kr_raw = [pkr.tile([128, TOK], F32, name=f"kr_raw{p}", tag=f"kr_raw{p}")
              for p in range(2)]

    with tc.tile_pool(name="w3a", bufs=1) as pw3, \
         tc.tile_pool(name="p3k", bufs=2, space="PSUM") as ppk:
        wu_ch = []
        for i in range(4):
            wt = pw3.tile([128, QROWS], F32R, name=f"wu{i}", tag=f"wu{i}")
            nc.sync.dma_start(wt[:], wuT[128 * i:128 * (i + 1), :])
            wu_ch.append(wt)
        k_out = [(kn[0][:], 0), (kn[1][:], 128), (kn[2][:], 256), (kn[3][:], 384),
                 (kr_raw[0][:], 512), (kr_raw[1][:], 640)]
        for t, (osb, coff) in enumerate(k_out):
            ps = ppk.tile([128, TOK], F32, name=f"p3k{t}", tag="p3k")
            for i in range(4):
                for s in range(4):
                    nc.tensor.matmul(ps[:, 512 * s:512 * (s + 1)],
                                     wu_ch[i][:, coff:coff + 128],
                                     c_nrm[i][:, 512 * s:512 * (s + 1)],
                                     start=(i == 0), stop=(i == 3))
            nc.vector.tensor_copy(osb, ps[:])
    with tc.tile_pool(name="ktmp", bufs=1) as pkt:
        for p in range(2):
            tmp = pkt.tile([128, TOK], F32, name=f"kropetmp{p}", tag="kropetmp")
            nc.vector.stream_shuffle(tmp[:], kr_raw[p][:], _SHUF_MASK)
            nc.vector.tensor_mul(tmp[:], tmp[:], sin_t[:])
            nc.vector.tensor_mul(kr_raw[p][:], kr_raw[p][:], cos_t[:])
            nc.vector.tensor_add(kr[p][:], kr_raw[p][:], tmp[:])
    pkr.release()
    pcs.release()

    pkv = tc.alloc_tile_pool(name="vpool", bufs=1, side="right")
    v_sb = [pkv.tile([128, NH * DV], F32R, name=f"v{t}", tag=f"v{t}")
            for t in range(16)]
    with tc.tile_pool(name="w3b", bufs=1) as pwv, \
         tc.tile_pool(name="p3v", bufs=8, space="PSUM") as ppv:
        wv_ch = []
        for i in range(4):
            vt = pwv.tile([128, NH * DV], F32R, name=f"wv{i}", tag=f"wv{i}")
            nc.sync.dma_start(vt[:], wvT[128 * i:128 * (i + 1), :])
            wv_ch.append(vt)
        for tt in range(16):
            vps = ppv.tile([128, NH * DV], F32, name=f"p3v{tt}", tag="p3v")
            for i in range(4):
                nc.tensor.matmul(vps[:],
                                 c_nrm[i][:, 128 * tt:128 * (tt + 1)],
                                 wv_ch[i][:],
                                 start=(i == 0), stop=(i == 3))
            nc.vector.tensor_copy(v_sb[tt][:], vps[:])
    pckv.release()

    po = tc.alloc_tile_pool(name="opool", bufs=1)
    o_sb = [[po.tile([128, 512], F32R, name=f"o{h}_{j}", tag=f"o{h}_{j}")
             for j in range(4)] for h in range(4)]

    pm = tc.alloc_tile_pool(name="mload", bufs=4)
    with tc.tile_pool(name="exp", bufs=4) as pe_, \
         tc.tile_pool(name="norm", bufs=4) as pn, \
         tc.tile_pool(name="qkps", bufs=2, space="PSUM") as pqk, \
         tc.tile_pool(name="pvps", bufs=2, space="PSUM") as ppv4, \
         tc.tile_pool(name="sumps", bufs=2, space="PSUM") as psum4, \
         tc.tile_pool(name="bcps", bufs=2, space="PSUM") as pbc4:
        for h in range(4):
            p = h // 2
            rs0 = 64 * (h % 2)
            for j in range(4):
                nch = 16
                pv_ps = ppv4.tile([128, 512], F32, name=f"pv{h}_{j}", tag="pv")
                sm_ps = psum4.tile([1, 512], F32, name=f"sm{h}_{j}", tag="sm")
                for ci in range(nch):
                    c = ci
                    qk_ps = pqk.tile([128, 512], F32, name=f"qk{h}_{j}_{c}", tag="qk")
                    nc.tensor.matmul(qk_ps[:],
                                     kn[h][:, 128 * c:128 * (c + 1)],
                                     qn[h][:, 512 * j:512 * (j + 1)],
                                     start=True, stop=False)
                    nc.tensor.matmul(qk_ps[:],
                                     kr[p][rs0:rs0 + 64, 128 * c:128 * (c + 1)],
                                     qr[p][rs0:rs0 + 64, 512 * j:512 * (j + 1)],
                                     start=False, stop=True)
                    mt = pm.tile([128, 512], F32, name=f"mt{h}{j}{c}", tag="mt")
                    nc.sync.dma_start(
                        mt[:], maskTd[128 * c:128 * (c + 1),
                                      512 * j:512 * (j + 1)])
                    nc.vector.tensor_add(qk_ps[:], qk_ps[:], mt[:])
                    e = pe_.tile([128, 512], F32R, name=f"e{h}{j}{c}", tag="e")
                    nc.scalar.activation(e[:], qk_ps[:], Exp)
                    nc.tensor.matmul(pv_ps[:],
                                     v_sb[c][:, 128 * h:128 * (h + 1)],
                                     e[:],
                                     start=(ci == 0), stop=(ci == nch - 1))
                    nc.tensor.matmul(sm_ps[:], ones_col[:], e[:],
                                     start=(ci == 0), stop=(ci == nch - 1))
                rr = pn.tile([1, 512], F32R, name=f"rr{h}{j}", tag="rr")
                nc.vector.reciprocal(rr[:], sm_ps[:])
                bc_ps = pbc4.tile([128, 512], F32, name=f"abc{h}{j}", tag="abc")
                nc.tensor.matmul(bc_ps[:], ones_row[:], rr[:], start=True, stop=True)
                rbc = pn.tile([128, 512], F32, name=f"rbc{h}{j}", tag="rbc")
                nc.vector.tensor_copy(rbc[:], bc_ps[:])
                nc.vector.tensor_mul(o_sb[h][j][:], pv_ps[:], rbc[:])
    pm.release()
    pkv.release()
    pkk.release()

    with tc.tile_pool(name="w5", bufs=1) as pw5, \
         tc.tile_pool(name="fout", bufs=3) as pf, \
         tc.tile_pool(name="p5ps", bufs=2, space="PSUM") as pps5:
        wo_ch = []
        for i in range(4):
            wt = pw5.tile([128, HID], F32R, name=f"wo{i}", tag=f"wo{i}")
            nc.sync.dma_start(wt[:], woT[128 * i:128 * (i + 1), :])
            wo_ch.append(wt)
        for dt in range(16):
            ps = pps5.tile([128, TOK], F32, name=f"p5_{dt}", tag="p5")
            for i in range(4):
                for j in range(4):
                    nc.tensor.matmul(ps[:, 512 * j:512 * (j + 1)],
                                     wo_ch[i][:, 128 * dt:128 * (dt + 1)],
                                     o_sb[i][j][:],
                                     start=(i == 0), stop=(i == 3))
            fo = pf.tile([128, TOK], F32, name=f"fo{dt}", tag="fo")
            nc.vector.tensor_copy(fo[:], ps[:])
            nc.sync.dma_start(outd[128 * dt:128 * (dt + 1), :], fo[:])
    po.release()
    pq.release()
    pp.release()


# ----------------------------------------------------------------------------
# Host-side input preparation
# ----------------------------------------------------------------------------

_ROPE_PERM = np.empty(DR, dtype=np.int64)
_ROPE_PERM[0::2] = np.arange(32)
_ROPE_PERM[1::2] = np.arange(32, 64)


def _reorder_headsT(w_shard):
    """[NH*QHD, X] head-major rows -> [X, QROWS] transposed, nope/rope-packed."""
    blocks = []
    for h in range(NH):
        rows = w_shard[h * QHD:(h + 1) * QHD]
        blocks.append(rows[:DN])
    for pair in range(2):
        for h in (2 * pair, 2 * pair + 1):
            rows = w_shard[h * QHD:(h + 1) * QHD]
            blocks.append(rows[DN:][_ROPE_PERM])
    w_re = np.concatenate(blocks, axis=0)  # [768, X]
    return np.ascontiguousarray(w_re.T).astype(np.float32)


def _build_dmask():
    dm = np.zeros((128, 4 * 512), dtype=np.float32)
    for d in range(4):
        for m in range(4):
            blk = dm[:, 512 * d + 128 * m: 512 * d + 128 * (m + 1)]
            if m < d:
                blk[:] = NEG
            elif m == d:
                kk = np.arange(128)[:, None]
                qq = np.arange(128)[None, :]
                blk[:] = np.where(kk > qq, NEG, 0.0)
    return dm


def _is_causal(mask):
    m = np.asarray(mask).reshape(S, S)
    iu = np.triu_indices(S, 1)
    if not np.all(m[iu] <= -1e8):
        return False
    il = np.tril_indices(S)
    return bool(np.all(m[il] == 0.0))


def _prep_in_maps(inputs):
    hidden = np.ascontiguousarray(np.asarray(inputs["hidden_states"], dtype=np.float32))
    mask = np.asarray(inputs["attention_mask"], dtype=np.float32)
    position_ids = np.asarray(inputs["position_ids"]).astype(np.int64)
    Wq = np.asarray(inputs["Wq"], dtype=np.float32)
    Wkv_down = np.asarray(inputs["Wkv_down"], dtype=np.float32)
    kv_norm_w = np.asarray(inputs["kv_norm_w"], dtype=np.float32)
    Wkv_up = np.asarray(inputs["Wkv_up"], dtype=np.float32)
    Wkv_v = np.asarray(inputs["Wkv_v"], dtype=np.float32)
    Wo = np.asarray(inputs["Wo"], dtype=np.float32)
    cos = np.asarray(inputs["cos"], dtype=np.float32)
    sin = np.asarray(inputs["sin"], dtype=np.float32)

    causal = _is_causal(mask)

    pos = position_ids.reshape(-1)[:S]
    cos_g = cos[pos]                      # [S, 64]
    sin_g = sin[pos]
    cosT = np.tile(np.ascontiguousarray(cos_g.T)[_ROPE_PERM], (2, 1)).astype(np.float32)
    sinP = np.ascontiguousarray(sin_g.T)[_ROPE_PERM].copy()
    sinP[0::2] = -sinP[0::2]              # row 2i (pairs with d+32): -sin
    sinT = np.tile(sinP, (2, 1)).astype(np.float32)

    wdT = np.ascontiguousarray(Wkv_down.T).astype(np.float32)
    Wkv_up_w = Wkv_up * kv_norm_w[None, :]
    Wkv_v_w = Wkv_v * kv_norm_w[None, :]

    dmask = _build_dmask() if causal else None
    maskT = None if causal else np.ascontiguousarray(mask.reshape(S, S).T)
    ident = np.eye(128, dtype=np.float32)

    in_maps = []
    for c in range(N_CORES):
        b, g = divmod(c, 4)
        heads = slice(g * NH * QHD, (g + 1) * NH * QHD)
        vh = slice(g * NH * DV, (g + 1) * NH * DV)
        wq_re = _reorder_headsT(Wq[heads] * np.float32(1.0 / math.sqrt(QHD)))
        m = {
            "hT": np.ascontiguousarray(hidden[b].T),
            "wuT": _reorder_headsT(Wkv_up_w[heads]),
            "wvT": np.ascontiguousarray(Wkv_v_w[vh].T).astype(np.float32),
            "woT": np.ascontiguousarray(Wo[:, vh].T).astype(np.float32),
            "cosd": cosT,
            "sind": sinT,
        }
        if causal:
            m["w0T"] = np.ascontiguousarray(wq_re[:, 0:512])
            m["w1T"] = np.ascontiguousarray(
                np.concatenate([wdT[:, 0:256], wq_re[:, 512:768]], axis=1))
            m["w2T"] = np.ascontiguousarray(wdT[:, 256:512])
            m["identd"] = ident
            m["dmaskd"] = dmask
        else:
            m["wqT"] = wq_re
            m["wdT"] = wdT
            m["maskTd"] = maskT
        in_maps.append(m)
    return causal, in_maps


def _combine(results):
    out = np.zeros((B, S, HID), dtype=np.float32)
    for b in range(B):
        acc = results[4 * b]["out"].astype(np.float32)
        for g in range(1, 4):
            acc = acc + results[4 * b + g]["out"]
        out[b] = acc.T
    return out


def kernel(**inputs):
    from concourse import bass_utils

    causal, in_maps = _prep_in_maps(inputs)
    if causal not in _CACHE:
        _CACHE[causal] = _build(causal)
    nc = _CACHE[causal]

    res = bass_utils.run_bass_kernel_spmd(nc, in_maps, core_ids=list(range(N_CORES)))
    return _combine(res.results)
